# revision 29
# baseline (speedup 1.0000x reference)
"""Trainium2 Bass kernel for nn_DecoupledTextDecoder.

Reference computation (per batch sample b, nB=256, nC=512, nH*nW=512, nT=40,
nCls=97):
  A_n   = A / sum_hw(A)                       (attention normalize)
  C     = einsum('chw,thw->tc', feature_b, A_n_b)       [40, 512]
  hidden= C @ W.T + b                                   [40, 512]
  cfP   = hidden @ protos.T                             [40, 97]
  cfCos = cfP / (||hidden||_row + EPS)
  outCls= concat([cfP * ALPHA, UNK], -1); outCos = concat([cfCos, UNK], -1)
  ragged-pack the first textLength[b] rows of each sample into one buffer.

Strategy: data-parallel over nB across 8 NeuronCores (32 samples/core).
Host-side algebraic folds shrink the device program to three matmul stages
and one activation pass:
  * The attention normalization is folded into A on host:
    at = fp8(256 * A / sum_hw(A)); the 1/256 is folded into W/protos.
  * M1: Craw^T[c,t] = ft-chunks(lhsT) x at-chunks(rhs), accum over hw.
    Both operands fp8 (e4m3) - halves the dominant HBM traffic.
  * M2: h0[t,c'] = ct-slices(lhsT) x (W.T/256)-chunks(rhs), accum over c.
    One Square-activation pass per row-group with accum_out gives
    ||h0||^2 per t-row directly (hidden itself is never stored).
  * M3: ct-slices(lhsT) x P2-chunks(rhs) where P2 = [(W.T@protos.T)/256,
    (W.T@b)/256]: col 0..96 = cfP-without-bias, col 97 = h0.b per row.
  * Host finishes: cfP += protos@b; hnorm^2 = h2 + 2*h0.b + ||b||^2;
    ALPHA/cos scaling, UNK column, and the ragged pack (pure data
    movement with runtime offsets) are all host-side.
All DRAM operands are host-pre-arranged so every DMA is contiguous per
partition (8KB+ descriptors); outputs consolidate into one [128, 11*99]
fp32 tensor written with 3 block-level DMAs.
"""

import numpy as np
import ml_dtypes

import concourse.bass as bass
import concourse.bacc as bacc
import concourse.tile as tile
import concourse.mybir as mybir
from concourse.bass_utils import run_bass_kernel_spmd

F32 = mybir.dt.float32
F16 = mybir.dt.float16
F8 = mybir.dt.float8e4
EPS = 0.0009

N_CORES = 8
NB = 256
NB_C = NB // N_CORES       # samples per core
NC = 512                   # channels
HW = 512                   # nH*nW
NT = 40                    # text steps
NCLS = 97
D = NCLS + 1

ASCALE = 256.0             # host scale on normalized A; 1/ASCALE folded into W
BLOCKS = [12, 12, 8]       # samples per block (sum = NB_C)
GROUP = 4                  # samples per feature DMA


def _mgroups(ns):
    """Row-groups of t-columns (<=120 each) for the per-group stages."""
    w = ns * NT
    out, o = [], 0
    while o < w:
        m = min(120, w - o)
        out.append((o, m))
        o += m
    return out


GTOT = sum(len(_mgroups(ns)) for ns in BLOCKS)   # 11 output groups
OCOLS = GTOT * (D + 1)                           # 99 cols per group


def build_kernel(reps=1, group=GROUP, timing_mode=False, hw_loop=0,
                 dt_in="f8", rings="sa", out_rings="sa", ft_bufs=5,
                 out_split=True, first_split=True,
                 skip_load=False, skip_compute=False, m1_dr=False,
                 out16=True, ct8=False):
    """Build + compile the per-core Bass program. Returns nc.

    timing_mode=True replaces the bulk inputs with Internal DRAM scratch so
    repeated-execution benchmarks don't pay host->device re-transfers; the
    on-device HBM traffic is identical.
    """
    nc = bacc.Bacc("TRN2", target_bir_lowering=False, debug=False,
                   enable_asserts=True, num_devices=N_CORES)
    dt_ft = {"f8": F8, "f16": F16}[dt_in]

    kind_b = "Internal" if timing_mode else "ExternalInput"
    dt_w = F8 if ct8 else F16
    ft = nc.dram_tensor("ft", [128, NB_C * 4 * NC], dt_ft, kind=kind_b).ap()
    at = nc.dram_tensor("at", [128, NB_C * 4 * NT], dt_ft, kind=kind_b).ap()
    wt = nc.dram_tensor("wt", [128, 4 * NC], dt_w, kind=kind_b).ap()
    p2 = nc.dram_tensor("p2", [128, 4 * D], dt_w, kind=kind_b).ap()
    if out16:
        oraw = nc.dram_tensor("oraw", [128, GTOT * D], F16,
                              kind="ExternalOutput").ap()
        oh2 = nc.dram_tensor("oh2", [128, GTOT], F32,
                             kind="ExternalOutput").ap()
    else:
        oraw = nc.dram_tensor("oraw", [128, OCOLS], F32,
                              kind="ExternalOutput").ap()
        oh2 = None

    with tile.TileContext(nc) as tc:
        with (
            tc.tile_pool(name="consts", bufs=1) as consts,
            tc.tile_pool(name="ftp", bufs=ft_bufs) as ftp,
            tc.tile_pool(name="ctp", bufs=8) as ctp,
            tc.tile_pool(name="sqp", bufs=2) as sqp,
            tc.tile_pool(name="outp", bufs=3) as outp,
            tc.tile_pool(name="ps_ct", bufs=4, space="PSUM") as ps_ct,
            tc.tile_pool(name="ps_h", bufs=2, space="PSUM") as ps_h,
            tc.tile_pool(name="ps_p", bufs=2, space="PSUM") as ps_p,
        ):
            emap = {"s": nc.sync, "a": nc.scalar, "g": nc.gpsimd}
            ring_eng = [emap[ch] for ch in rings]
            oring_eng = [emap[ch] for ch in out_rings]

            def emit():
                _emit_once(nc, tc, consts, ftp, ctp, sqp, outp,
                           ps_ct, ps_h, ps_p,
                           ft, at, wt, p2, oraw, oh2, dt_ft, dt_w,
                           group, ring_eng, oring_eng, out_split, first_split,
                           skip_load, skip_compute, m1_dr, out16)

            if hw_loop:
                with tc.For_i(0, hw_loop, 1):
                    emit()
            else:
                for _ in range(reps):
                    emit()
    nc.compile()
    return nc


def _emit_once(nc, tc, consts, ftp, ctp, sqp, outp,
               ps_ct, ps_h, ps_p,
               ft, at, wt, p2, oraw, oh2, dt_ft, dt_w, group, ring_eng,
               oring_eng, out_split=True, first_split=True,
               skip_load=False, skip_compute=False, m1_dr=False, out16=True):
    nring = len(ring_eng)
    ow = D if out16 else D + 1       # output cols per group
    dt_o = F16 if out16 else F32

    # ---- input loads, startup-ordered -----------------------------------
    # Issue order per queue matters: the first feature tile and the first
    # half of the attention maps gate the first M1, so they go first on
    # separate queues; wt/p2 are not needed until M2 (~halfway) and load
    # behind the early feature tiles.
    n_ft = sum(ns // group for ns in BLOCKS)
    ft_tiles = [ftp.tile([128, group * 4, NC], dt_ft, tag="ft",
                         name=f"ftile{j}") for j in range(n_ft)]
    ft_offs = []
    for bi, ns in enumerate(BLOCKS):
        s_base = sum(BLOCKS[:bi])
        for j in range(ns // group):
            ft_offs.append((s_base + j * group) * 4 * NC)

    at_all = consts.tile([128, NB_C * 4, NT], dt_ft, tag="at_all")
    wt_sb = consts.tile([128, 4 * NC], dt_w, tag="wt")
    p2_sb = consts.tile([128, 4 * D], dt_w, tag="p2")
    h2sb = None
    if out16:
        h2sb = consts.tile([128, GTOT], F32, tag="h2sb", name="h2sb")
    half = NB_C * 2
    gsz = group * 4 * NC

    def load_ft(j, eng):
        if j == 0 and first_split:
            h = gsz // 2
            eng.dma_start(out=ft_tiles[0][:, :group * 2, :],
                          in_=ft[:, :h])
            eng.dma_start(out=ft_tiles[0][:, group * 2:, :],
                          in_=ft[:, h:gsz])
        else:
            off = ft_offs[j]
            eng.dma_start(out=ft_tiles[j][:], in_=ft[:, off:off + gsz])

    issue = [
        (0, lambda e: load_ft(0, e)),
        (1 % nring, lambda e: e.dma_start(out=at_all[:, :half, :],
                                          in_=at[:, :half * NT])),
        (1 % nring, lambda e: load_ft(1, e)),
        (2 % nring, lambda e: load_ft(2, e)),
        (0, lambda e: e.dma_start(out=wt_sb[:], in_=wt[:])),
        (1 % nring, lambda e: e.dma_start(out=p2_sb[:], in_=p2[:])),
        (2 % nring, lambda e: e.dma_start(out=at_all[:, half:, :],
                                          in_=at[:, half * NT:])),
    ]
    for j in range(3, n_ft):
        issue.append((j % nring, lambda e, j=j: load_ft(j, e)))
    if not skip_load:
        for r, thunk in issue:
            thunk(ring_eng[r])
    if skip_compute:
        # dummy readers keep the tile framework honest; ~17ns each on PE
        dps = ps_p.tile([128, D], F32, tag="p")
        for j in range(n_ft):
            nc.tensor.matmul(dps[:, :NT], ft_tiles[j][:, 0, 0:128],
                             at_all[:, 0, :], start=True, stop=True)
        nc.tensor.matmul(dps[:1, :D], wt_sb[:, 0:1], p2_sb[:, :D],
                         start=True, stop=True)
        od = outp.tile([128, 4 * ow], dt_o, tag="od")
        nc.vector.tensor_copy(od[:, :D], dps[:, :D])
        oring_eng[0].dma_start(out=oraw[:, :D], in_=od[:, :D])
        return

    # ---- main loop over sample blocks -----------------------------------
    s0 = 0
    gi = 0          # global output-group index
    fj = 0          # feature-tile index
    for ns in BLOCKS:
        w = ns * NT
        ftg = [ft_tiles[fj + j] for j in range(ns // group)]
        fj += ns // group

        # M1: Craw^T accumulated into 4 psum banks, one 40-col slice/sample
        ct_ps = [ps_ct.tile([128, 512], F32, tag="ct", name=f"ct_ps{jj}")
                 for jj in range(4)]
        for sl in range(ns):
            ftile = ftg[sl // group]
            h = sl % group
            abase = (s0 + sl) * 4
            for jj in range(4):
                if m1_dr:
                    # fp8 double-row: two hw-chunks per matmul (pair-major
                    # along the free dims of both operands)
                    for P in range(2):
                        nc.tensor.matmul(
                            ct_ps[jj][:, sl * NT:(sl + 1) * NT],
                            ftile[:, h * 4 + 2 * P:h * 4 + 2 * P + 2,
                                  jj * 128:(jj + 1) * 128],
                            at_all[:, abase + 2 * P:abase + 2 * P + 2, :],
                            perf_mode=mybir.MatmulPerfMode.DoubleRow,
                            start=(P == 0), stop=(P == 1))
                else:
                    for kk in range(4):
                        nc.tensor.matmul(
                            ct_ps[jj][:, sl * NT:(sl + 1) * NT],
                            ftile[:, h * 4 + kk, jj * 128:(jj + 1) * 128],
                            at_all[:, abase + kk, :],
                            start=(kk == 0), stop=(kk == 3))

        # cast to fp16/fp8 for the downstream matmuls
        ct_sb = []
        for jj in range(4):
            t = ctp.tile([128, 480], dt_w, tag="ct_sb")
            nc.vector.tensor_copy(t[:, :w], ct_ps[jj][:, :w])
            ct_sb.append(t)

        odense = outp.tile([128, 4 * ow], dt_o, tag="od")
        ng = 0
        for (o, m) in _mgroups(ns):
            col = ng * ow
            # M2: h0[t, c'] for this row-group; only its row-norms survive
            h2_ps = ps_h.tile([128, 512], F32, tag="h2")
            for kk in range(4):
                nc.tensor.matmul(h2_ps[:m, :], ct_sb[kk][:, o:o + m],
                                 wt_sb[:, kk * NC:(kk + 1) * NC],
                                 start=(kk == 0), stop=(kk == 3))
            sq = sqp.tile([128, 512], F32, tag="sq")
            acc = (h2sb[:m, gi + ng:gi + ng + 1] if out16
                   else odense[:m, col + D:col + D + 1])
            nc.scalar.activation(sq[:m, :], h2_ps[:m, :],
                                 mybir.ActivationFunctionType.Square,
                                 accum_out=acc)

            # M3: cfP-without-bias (97 cols) + h0.b (col 97)
            p_ps = ps_p.tile([128, D], F32, tag="p")
            for kk in range(4):
                nc.tensor.matmul(p_ps[:m, :], ct_sb[kk][:, o:o + m],
                                 p2_sb[:, kk * D:(kk + 1) * D],
                                 start=(kk == 0), stop=(kk == 3))
            nc.vector.tensor_copy(odense[:m, col:col + D], p_ps[:m, :])
            ng += 1

        if out_split:
            done = 0
            while done < ng:
                take = min(2, ng - done)
                c0 = done * ow
                oring_eng[(gi + done) % len(oring_eng)].dma_start(
                    out=oraw[:, (gi + done) * ow:(gi + done + take) * ow],
                    in_=odense[:, c0:c0 + take * ow])
                done += take
        else:
            oring_eng[gi % len(oring_eng)].dma_start(
                out=oraw[:, gi * ow:(gi + ng) * ow],
                in_=odense[:, :ng * ow])
        gi += ng
        s0 += ns
    if out16:
        oring_eng[-1].dma_start(out=oh2[:], in_=h2sb[:])


# ---- host side -----------------------------------------------------------

def host_prep(feature, A, protos, W, b, np_dt=ml_dtypes.float8_e4m3,
              ct8=False):
    """Build the 8 per-core input maps (host-side layout + algebra prep)."""
    w_dt = ml_dtypes.float8_e4m3 if ct8 else np.float16
    f3 = feature.reshape(NB, NC, HW).transpose(0, 2, 1)      # [NB, HW, NC]
    a3 = A.reshape(NB, NT, HW)
    s = a3.sum(axis=2, dtype=np.float64)                     # [NB, NT]
    a_n = (ASCALE * a3 / s[:, :, None]).astype(np.float32)
    a_n = a_n.transpose(0, 2, 1)                             # [NB, HW, NT]

    wt2 = (W.T.astype(np.float64) / ASCALE)                  # [c, c']
    wt_dev = wt2.reshape(4, 128, NC).transpose(1, 0, 2).reshape(128, 4 * NC)
    p2t = wt2 @ protos.T.astype(np.float64)                  # [c, 97]
    hbcol = wt2 @ b.astype(np.float64)                       # [c]
    p2full = np.concatenate([p2t, hbcol[:, None]], axis=1)   # [c, 98]
    p2_dev = p2full.reshape(4, 128, D).transpose(1, 0, 2).reshape(128, 4 * D)

    in_maps = []
    for c in range(N_CORES):
        sl = slice(c * NB_C, (c + 1) * NB_C)
        fc = f3[sl].reshape(NB_C, 4, 128, NC).transpose(2, 0, 1, 3)
        ac = a_n[sl].reshape(NB_C, 4, 128, NT).transpose(2, 0, 1, 3)
        in_maps.append(dict(
            ft=np.ascontiguousarray(fc.reshape(128, NB_C * 4 * NC)).astype(np_dt),
            at=np.ascontiguousarray(ac.reshape(128, NB_C * 4 * NT)).astype(np_dt),
            wt=wt_dev.astype(w_dt),
            p2=p2_dev.astype(w_dt),
        ))
    return in_maps


def host_finish(oraws, oh2s, protos, W, b, ALPHA, UNK_SCR):
    """Decode the per-core raw outputs into dense [NB*NT, D] cls/cos.

    oh2s is the list of per-core [128, GTOT] fp32 row-norm tensors (out16
    layout); if None, oraws carry fp32 [128, GTOT*(D+1)] with h2 inline.
    """
    pb = (protos.astype(np.float64) @ b.astype(np.float64)).astype(np.float32)
    bb = float(b.astype(np.float64) @ b.astype(np.float64))
    alpha = float(np.asarray(ALPHA).reshape(-1)[0])
    unk = float(np.asarray(UNK_SCR).reshape(-1)[0])

    groups = []     # (flat_row_base, m, gi)
    s0, gi = 0, 0
    for ns in BLOCKS:
        for (o, m) in _mgroups(ns):
            groups.append((s0 * NT + o, m, gi))
            gi += 1
        s0 += ns

    dense_cls = np.empty((NB * NT, D), np.float32)
    dense_cos = np.empty((NB * NT, D), np.float32)
    for c, raw in enumerate(oraws):
        ncols = D if oh2s is not None else D + 1
        r = raw.reshape(128, GTOT, ncols).astype(np.float32)
        base_c = c * NB_C * NT
        for row0, m, g in groups:
            cf = r[:m, g, :NCLS] + pb[None, :]
            hb = r[:m, g, NCLS]
            h2 = (oh2s[c][:m, g] if oh2s is not None else r[:m, g, NCLS + 1])
            hnorm = np.sqrt(np.maximum(h2 + 2.0 * hb + bb, 0.0))
            rows = slice(base_c + row0, base_c + row0 + m)
            dense_cls[rows, :NCLS] = cf * alpha
            dense_cos[rows, :NCLS] = cf / (hnorm[:, None] + EPS)
    dense_cls[:, NCLS] = unk
    dense_cos[:, NCLS] = unk
    return dense_cls, dense_cos


def host_pack(dense_cls, dense_cos, textLength):
    """Ragged per-sample packing (matches reference.pack)."""
    usedLen = np.minimum(textLength.astype(np.int64), NT)
    offsets = np.cumsum(usedLen) - usedLen
    b_idx, t_idx = np.nonzero(np.arange(NT)[None, :] < usedLen[:, None])
    out_cls = np.zeros((NB * NT, D), np.float32)
    out_cos = np.zeros((NB * NT, D), np.float32)
    dest = offsets[b_idx] + t_idx
    src = b_idx * NT + t_idx
    out_cls[dest] = dense_cls[src]
    out_cos[dest] = dense_cos[src]
    return out_cls, out_cos


_NC_CACHE = {}


def _get_nc(**kw):
    key = tuple(sorted(kw.items()))
    if key not in _NC_CACHE:
        _NC_CACHE[key] = build_kernel(**kw)
    return _NC_CACHE[key]


FINAL_CFG = dict(dt_in="f8", rings="sa", out_rings="sa")


def kernel(feature, A, protos, W, b, ALPHA, UNK_SCR, textLength):
    feature = np.asarray(feature, np.float32)
    A = np.asarray(A, np.float32)
    protos = np.asarray(protos, np.float32)
    W = np.asarray(W, np.float32)
    b = np.asarray(b, np.float32)
    in_maps = host_prep(feature, A, protos, W, b,
                        ct8=FINAL_CFG.get("ct8", False))
    nc = _get_nc(**FINAL_CFG)
    res = None
    for attempt in range(3):
        try:
            res = run_bass_kernel_spmd(nc, in_maps, core_ids=list(range(N_CORES)))
            break
        except Exception:  # noqa: BLE001 - transient device/tunnel hiccups
            if attempt == 2:
                raise
            import time as _time
            _time.sleep(30)
    oraws = [res.results[c]["oraw"] for c in range(N_CORES)]
    oh2s = ([res.results[c]["oh2"] for c in range(N_CORES)]
            if FINAL_CFG.get("out16", True) else None)
    dense_cls, dense_cos = host_finish(oraws, oh2s, protos, W, b,
                                       ALPHA, UNK_SCR)
    return host_pack(dense_cls, dense_cos, np.asarray(textLength))
